# revision 4
# baseline (speedup 1.0000x reference)
"""HEX loss kernel for Trainium2 (8 NeuronCores, batch-parallel).

Math: the chain junction tree potential factorizes per variable
(pot[c,i,j] = exp(s_i*a_c) * exp(s_j*b_c), and each interior fs[v] is
split fs[v]/2 over its two cliques), so the joint distribution is a
product of independent Bernoullis with P(y_v=1) = sigmoid(fs[v]).
Hence pMargin[labels[b], b] = sigmoid(fs[b, labels[b]]) and

    loss = mean_b softplus(-fs[b, labels[b]])

(verified to 1.4e-16 rel err against the f64 reference).

Device work per core (B_loc = 4096 rows): stream fs (4 MB), select
fs[b, labels[b]] via an iota==label mask, multiply on DVE, row-sum via
ACT Copy+accum. The gen3 ACT tables have no Ln/Softplus, so
softplus(-x) = ln(z), z = 1 + exp(-x), is computed as: root-doubling
initial guess (w = z^(1/256) via 8 chained Sqrts, 4-term series in
w - 1, scaled by 256), then 2 Newton steps y' = y + z*exp(-y) - 1,
whose accuracy is set by one unamplified Exp table call (~1e-5).
Per-core partial sums [128, 1] come back; host sums / B.
"""

import numpy as np

B = 32768
V = 256
N_CORES = 8
BL = B // N_CORES          # 4096 rows per core
P = 128                    # SBUF partitions
RPP = 8                    # rows per partition per DMA group
GROUP_ROWS = P * RPP       # 1024 rows per group (1 MB)
N_GROUPS = BL // GROUP_ROWS  # 4
NT = BL // P               # 32 selected values per partition
N_SQRT = 8
LOG_SCALE = float(1 << N_SQRT)  # 256
N_NEWTON = 2

_CACHE = {}


def _build():
    from contextlib import ExitStack

    import concourse.bass as bass
    import concourse.tile as tile
    from concourse import bacc, mybir

    f32 = mybir.dt.float32
    Alu = mybir.AluOpType
    Act = mybir.ActivationFunctionType

    nc = bacc.Bacc(
        "TRN2",
        target_bir_lowering=False,
        debug=False,
        enable_asserts=True,
        num_devices=N_CORES,
    )

    fs_d = nc.dram_tensor("fs", [BL, V], f32, kind="ExternalInput").ap()
    lab_d = nc.dram_tensor("labt", [P, NT], f32, kind="ExternalInput").ap()
    out_d = nc.dram_tensor("out", [P, 1], f32, kind="ExternalOutput").ap()

    with tile.TileContext(nc) as tc, ExitStack() as ctx:
        const_pool = ctx.enter_context(tc.tile_pool(name="const", bufs=1))
        fs_pool = ctx.enter_context(tc.tile_pool(name="fs", bufs=3))
        mask_pool = ctx.enter_context(tc.tile_pool(name="mask", bufs=4))
        scratch_pool = ctx.enter_context(tc.tile_pool(name="scratch", bufs=4))
        ep_pool = ctx.enter_context(tc.tile_pool(name="ep", bufs=2))

        iota = const_pool.tile([P, V], f32)
        nc.gpsimd.iota(
            iota[:],
            pattern=[[1, V]],
            base=0,
            channel_multiplier=0,
            allow_small_or_imprecise_dtypes=True,
        )

        labt = const_pool.tile([P, NT], f32)
        nc.sync.dma_start(out=labt[:], in_=lab_d[:])

        sel = const_pool.tile([P, NT], f32, tag="sel")

        # fs rows g*1024 .. (g+1)*1024, partition p <- 8 consecutive rows
        fs_view = fs_d.rearrange("(g p j) v -> g p (j v)", g=N_GROUPS, p=P, j=RPP)

        for g in range(N_GROUPS):
            fst = fs_pool.tile([P, RPP * V], f32)
            nc.sync.dma_start(out=fst[:], in_=fs_view[g])
            for j in range(RPP):
                t = g * RPP + j
                mask = mask_pool.tile([P, V], f32)
                nc.gpsimd.tensor_scalar(
                    mask[:],
                    iota[:],
                    labt[:, t : t + 1],
                    None,
                    Alu.is_equal,
                )
                scratch = scratch_pool.tile([P, V], f32)
                nc.vector.tensor_mul(scratch[:], fst[:, j * V : (j + 1) * V], mask[:])
                junk = scratch_pool.tile([P, V], f32, tag="junk")
                nc.scalar.activation(
                    junk[:],
                    scratch[:],
                    Act.Copy,
                    accum_out=sel[:, t : t + 1],
                )

        # epilogue: y[p,t] = softplus(-sel[p,t]); out[p] = sum_t y[p,t]
        # u = exp(-sel); z = 1 + u
        u = ep_pool.tile([P, NT], f32, tag="u")
        nc.scalar.activation(u[:], sel[:], Act.Exp, scale=-1.0)
        z = ep_pool.tile([P, NT], f32, tag="z")
        nc.vector.tensor_scalar(z[:], u[:], 1.0, None, Alu.add)
        # initial guess: w = z^(1/256) via 8 sqrts; s = w-1;
        # y0 = 256 * s * (1 - s*(1/2 - s*(1/3 - s/4)))
        w = ep_pool.tile([P, NT], f32, tag="w")
        nc.scalar.activation(w[:], u[:], Act.Sqrt, bias=1.0)
        for _ in range(N_SQRT - 1):
            w2 = ep_pool.tile([P, NT], f32, tag="w")
            nc.scalar.activation(w2[:], w[:], Act.Sqrt)
            w = w2
        s = ep_pool.tile([P, NT], f32, tag="s")
        nc.vector.tensor_scalar(s[:], w[:], -1.0, None, Alu.add)
        c = ep_pool.tile([P, NT], f32, tag="c")
        nc.vector.tensor_scalar(c[:], s[:], -0.25, 1.0 / 3.0, Alu.mult, Alu.add)
        sc = ep_pool.tile([P, NT], f32, tag="sc")
        nc.vector.tensor_mul(sc[:], s[:], c[:])
        d = ep_pool.tile([P, NT], f32, tag="d")
        nc.vector.tensor_scalar(d[:], sc[:], -1.0, 0.5, Alu.mult, Alu.add)
        sd = ep_pool.tile([P, NT], f32, tag="sd")
        nc.vector.tensor_mul(sd[:], s[:], d[:])
        e = ep_pool.tile([P, NT], f32, tag="e")
        nc.vector.tensor_scalar(e[:], sd[:], -1.0, 1.0, Alu.mult, Alu.add)
        se = ep_pool.tile([P, NT], f32, tag="se")
        nc.vector.tensor_mul(se[:], s[:], e[:])
        y = ep_pool.tile([P, NT], f32, tag="y")
        nc.vector.tensor_scalar(y[:], se[:], LOG_SCALE, None, Alu.mult)
        # Newton: y' = y + z*exp(-y) - 1   (fixed point: exp(y)=z)
        for it in range(N_NEWTON):
            ey = ep_pool.tile([P, NT], f32, tag="ey")
            nc.scalar.activation(ey[:], y[:], Act.Exp, scale=-1.0)
            pz = ep_pool.tile([P, NT], f32, tag="pz")
            nc.vector.tensor_mul(pz[:], z[:], ey[:])
            q = ep_pool.tile([P, NT], f32, tag="q")
            nc.vector.tensor_add(q[:], y[:], pz[:])
            y2 = ep_pool.tile([P, NT], f32, tag="y")
            nc.vector.tensor_scalar(y2[:], q[:], -1.0, None, Alu.add)
            y = y2
        acc = ep_pool.tile([P, 1], f32, tag="acc")
        jep = ep_pool.tile([P, NT], f32, tag="jep")
        nc.scalar.activation(jep[:], y[:], Act.Copy, accum_out=acc[:])
        nc.sync.dma_start(out=out_d[:], in_=acc[:])

    nc.compile()
    return nc


def _get_nc():
    if "nc" not in _CACHE:
        _CACHE["nc"] = _build()
    return _CACHE["nc"]


def _shard_inputs(fs, labels):
    fs = np.ascontiguousarray(np.asarray(fs, dtype=np.float32))
    labels = np.asarray(labels)
    in_maps = []
    for c in range(N_CORES):
        fs_loc = fs[c * BL : (c + 1) * BL]
        lab_loc = labels[c * BL : (c + 1) * BL]
        # labt[p, g*RPP + j] = lab_loc[g*GROUP_ROWS + p*RPP + j]
        labt = (
            lab_loc.reshape(N_GROUPS, P, RPP)
            .transpose(1, 0, 2)
            .reshape(P, NT)
            .astype(np.float32)
        )
        in_maps.append({"fs": fs_loc, "labt": np.ascontiguousarray(labt)})
    return in_maps


def kernel(fs, labels, _trace=False, _trace_kwargs=None):
    from concourse.bass_utils import run_bass_kernel_spmd

    nc = _get_nc()
    in_maps = _shard_inputs(fs, labels)
    res = run_bass_kernel_spmd(
        nc,
        in_maps,
        core_ids=list(range(N_CORES)),
        trace=_trace,
        **(_trace_kwargs or {}),
    )
    total = np.float64(0.0)
    for c in range(N_CORES):
        total += res.results[c]["out"].astype(np.float64).sum()
    loss = total / np.float64(B)
    if _trace:
        return np.float64(loss), res
    return np.asarray(loss, dtype=np.float64)


# revision 5
# speedup vs baseline: 3.0895x; 3.0895x over previous
"""HEX loss kernel for Trainium2 (8 NeuronCores, batch-parallel).

Math: the chain junction tree potential factorizes per variable
(pot[c,i,j] = exp(s_i*a_c) * exp(s_j*b_c), and each interior fs[v] is
split fs[v]/2 over its two cliques), so the joint distribution is a
product of independent Bernoullis with P(y_v=1) = sigmoid(fs[v]).
Hence pMargin[labels[b], b] = sigmoid(fs[b, labels[b]]) and

    loss = mean_b softplus(-fs[b, labels[b]])

(verified to 1.4e-16 rel err against the f64 reference).

Device work per core (B_loc = 4096 rows): stream fs (4 MB). The gather
fs[b, labels[b]] uses an exact max-trick: ACT computes
penalty = Square(10*iota - 10*lab) = 100*(v - lab)^2 (exact in f32 --
all intermediates < 2^24), DVE computes t = fs - penalty, and a grouped
reduce_max gives sel = max_v t = fs[b, lab] exactly (|fs| < 100).
The gen3 ACT tables have no Ln/Softplus, so softplus(-x) = ln(z),
z = 1 + exp(-x), is computed as: root-doubling initial guess
(w = z^(1/256) via 8 chained Sqrts, 4-term series in w - 1, scaled by
256), then 2 Newton steps y' = y + z*exp(-y) - 1, whose accuracy is
set by one unamplified Exp table call (~1e-5).
Per-core partial sums [128, 1] come back; host sums / B.
"""

import numpy as np

B = 32768
V = 256
N_CORES = 8
BL = B // N_CORES          # 4096 rows per core
P = 128                    # SBUF partitions
RPP = 8                    # rows per partition per DMA group
GROUP_ROWS = P * RPP       # 1024 rows per group (1 MB)
N_GROUPS = BL // GROUP_ROWS  # 4
NT = BL // P               # 32 selected values per partition
N_SQRT = 8
LOG_SCALE = float(1 << N_SQRT)  # 256
N_NEWTON = 2
PEN = 10.0                 # penalty scale sqrt: penalty = (PEN*d)^2

_CACHE = {}


def _build():
    from contextlib import ExitStack

    import concourse.bass as bass
    import concourse.tile as tile
    from concourse import bacc, mybir

    f32 = mybir.dt.float32
    Alu = mybir.AluOpType
    Act = mybir.ActivationFunctionType

    nc = bacc.Bacc(
        "TRN2",
        target_bir_lowering=False,
        debug=False,
        enable_asserts=True,
        num_devices=N_CORES,
    )

    fs_d = nc.dram_tensor("fs", [BL, V], f32, kind="ExternalInput").ap()
    lab_d = nc.dram_tensor("labt", [P, NT], f32, kind="ExternalInput").ap()
    out_d = nc.dram_tensor("out", [P, 1], f32, kind="ExternalOutput").ap()

    with tile.TileContext(nc) as tc, ExitStack() as ctx:
        const_pool = ctx.enter_context(tc.tile_pool(name="const", bufs=1))
        fs_pool = ctx.enter_context(tc.tile_pool(name="fs", bufs=3))
        sq_pool = ctx.enter_context(tc.tile_pool(name="sq", bufs=6))
        prod_pool = ctx.enter_context(tc.tile_pool(name="prod", bufs=2))
        ep_pool = ctx.enter_context(tc.tile_pool(name="ep", bufs=2))

        iota = const_pool.tile([P, V], f32)
        nc.gpsimd.iota(
            iota[:],
            pattern=[[1, V]],
            base=0,
            channel_multiplier=0,
            allow_small_or_imprecise_dtypes=True,
        )

        labt = const_pool.tile([P, NT], f32)
        nc.sync.dma_start(out=labt[:], in_=lab_d[:])
        # bias for ACT Square: -PEN * lab per partition
        labp = const_pool.tile([P, NT], f32, tag="labp")
        nc.vector.tensor_scalar(labp[:], labt[:], -PEN, None, Alu.mult)

        sel = const_pool.tile([P, NT], f32, tag="sel")

        # fs rows g*1024 .. (g+1)*1024, partition p <- 8 consecutive rows
        fs_view = fs_d.rearrange("(g p j) v -> g p (j v)", g=N_GROUPS, p=P, j=RPP)

        for g in range(N_GROUPS):
            fst = fs_pool.tile([P, RPP * V], f32)
            nc.sync.dma_start(out=fst[:], in_=fs_view[g])
            prod = prod_pool.tile([P, RPP * V], f32)
            for j in range(RPP):
                t = g * RPP + j
                # penalty = (PEN*iota - PEN*lab)^2 = PEN^2 * (v - lab)^2
                sq = sq_pool.tile([P, V], f32)
                nc.scalar.activation(
                    sq[:],
                    iota[:],
                    Act.Square,
                    scale=PEN,
                    bias=labp[:, t : t + 1],
                )
                # t = fs - penalty  (== fs at v==lab, <= fs-88 elsewhere)
                nc.vector.tensor_sub(
                    prod[:, j * V : (j + 1) * V],
                    fst[:, j * V : (j + 1) * V],
                    sq[:],
                )
            # sel[:, g*8:(g+1)*8] = max over v of prod
            nc.vector.tensor_reduce(
                sel[:, g * RPP : (g + 1) * RPP],
                prod[:].rearrange("p (j v) -> p j v", j=RPP),
                axis=mybir.AxisListType.X,
                op=Alu.max,
            )

        # epilogue: y[p,t] = softplus(-sel[p,t]); out[p] = sum_t y[p,t]
        # u = exp(-sel); z = 1 + u
        u = ep_pool.tile([P, NT], f32, tag="u")
        nc.scalar.activation(u[:], sel[:], Act.Exp, scale=-1.0)
        z = ep_pool.tile([P, NT], f32, tag="z")
        nc.vector.tensor_scalar(z[:], u[:], 1.0, None, Alu.add)
        # initial guess: w = z^(1/256) via 8 sqrts; s = w-1;
        # y0 = 256 * s * (1 - s*(1/2 - s*(1/3 - s/4)))
        w = ep_pool.tile([P, NT], f32, tag="w")
        nc.scalar.activation(w[:], u[:], Act.Sqrt, bias=1.0)
        for _ in range(N_SQRT - 1):
            w2 = ep_pool.tile([P, NT], f32, tag="w")
            nc.scalar.activation(w2[:], w[:], Act.Sqrt)
            w = w2
        s = ep_pool.tile([P, NT], f32, tag="s")
        nc.vector.tensor_scalar(s[:], w[:], -1.0, None, Alu.add)
        c = ep_pool.tile([P, NT], f32, tag="c")
        nc.vector.tensor_scalar(c[:], s[:], -0.25, 1.0 / 3.0, Alu.mult, Alu.add)
        sc = ep_pool.tile([P, NT], f32, tag="sc")
        nc.vector.tensor_mul(sc[:], s[:], c[:])
        d = ep_pool.tile([P, NT], f32, tag="d")
        nc.vector.tensor_scalar(d[:], sc[:], -1.0, 0.5, Alu.mult, Alu.add)
        sd = ep_pool.tile([P, NT], f32, tag="sd")
        nc.vector.tensor_mul(sd[:], s[:], d[:])
        e = ep_pool.tile([P, NT], f32, tag="e")
        nc.vector.tensor_scalar(e[:], sd[:], -1.0, 1.0, Alu.mult, Alu.add)
        se = ep_pool.tile([P, NT], f32, tag="se")
        nc.vector.tensor_mul(se[:], s[:], e[:])
        y = ep_pool.tile([P, NT], f32, tag="y")
        nc.vector.tensor_scalar(y[:], se[:], LOG_SCALE, None, Alu.mult)
        # Newton: y' = y + z*exp(-y) - 1   (fixed point: exp(y)=z)
        for it in range(N_NEWTON):
            ey = ep_pool.tile([P, NT], f32, tag="ey")
            nc.scalar.activation(ey[:], y[:], Act.Exp, scale=-1.0)
            pz = ep_pool.tile([P, NT], f32, tag="pz")
            nc.vector.tensor_mul(pz[:], z[:], ey[:])
            q = ep_pool.tile([P, NT], f32, tag="q")
            nc.vector.tensor_add(q[:], y[:], pz[:])
            y2 = ep_pool.tile([P, NT], f32, tag="y")
            nc.vector.tensor_scalar(y2[:], q[:], -1.0, None, Alu.add)
            y = y2
        acc = ep_pool.tile([P, 1], f32, tag="acc")
        jep = ep_pool.tile([P, NT], f32, tag="jep")
        nc.scalar.activation(jep[:], y[:], Act.Copy, accum_out=acc[:])
        nc.sync.dma_start(out=out_d[:], in_=acc[:])

    nc.compile()
    return nc


def _get_nc():
    if "nc" not in _CACHE:
        _CACHE["nc"] = _build()
    return _CACHE["nc"]


def _shard_inputs(fs, labels):
    fs = np.ascontiguousarray(np.asarray(fs, dtype=np.float32))
    labels = np.asarray(labels)
    in_maps = []
    for c in range(N_CORES):
        fs_loc = fs[c * BL : (c + 1) * BL]
        lab_loc = labels[c * BL : (c + 1) * BL]
        # labt[p, g*RPP + j] = lab_loc[g*GROUP_ROWS + p*RPP + j]
        labt = (
            lab_loc.reshape(N_GROUPS, P, RPP)
            .transpose(1, 0, 2)
            .reshape(P, NT)
            .astype(np.float32)
        )
        in_maps.append({"fs": fs_loc, "labt": np.ascontiguousarray(labt)})
    return in_maps


def kernel(fs, labels, _trace=False, _trace_kwargs=None):
    from concourse.bass_utils import run_bass_kernel_spmd

    nc = _get_nc()
    in_maps = _shard_inputs(fs, labels)
    res = run_bass_kernel_spmd(
        nc,
        in_maps,
        core_ids=list(range(N_CORES)),
        trace=_trace,
        **(_trace_kwargs or {}),
    )
    total = np.float64(0.0)
    for c in range(N_CORES):
        total += res.results[c]["out"].astype(np.float64).sum()
    loss = total / np.float64(B)
    if _trace:
        return np.float64(loss), res
    return np.asarray(loss, dtype=np.float64)


# revision 6
# speedup vs baseline: 3.7288x; 1.2069x over previous
"""HEX loss kernel for Trainium2 (8 NeuronCores, batch-parallel).

Math: the chain junction tree potential factorizes per variable
(pot[c,i,j] = exp(s_i*a_c) * exp(s_j*b_c), and each interior fs[v] is
split fs[v]/2 over its two cliques), so the joint distribution is a
product of independent Bernoullis with P(y_v=1) = sigmoid(fs[v]).
Hence pMargin[labels[b], b] = sigmoid(fs[b, labels[b]]) and

    loss = mean_b softplus(-fs[b, labels[b]])

(verified to 1.4e-16 rel err against the f64 reference).

Device work per core (B_loc = 4096 rows): stream fs (4 MB). The gather
fs[b, labels[b]] uses an exact max-trick: ACT computes
penalty = Square(10*iota - 10*lab) = 100*(v - lab)^2 (exact in f32 --
all intermediates < 2^24), DVE computes t = fs - penalty, and a grouped
reduce_max gives sel = max_v t = fs[b, lab] exactly (|fs| < 100).
The gen3 ACT tables have no Ln/Softplus, so softplus(-x) = ln(z),
z = 1 + exp(-x), is computed as: root-doubling initial guess
(w = z^(1/256) via 8 chained Sqrts, 4-term series in w - 1, scaled by
256), then 2 Newton steps y' = y + z*exp(-y) - 1, whose accuracy is
set by one unamplified Exp table call (~1e-5).
Per-core partial sums [128, 1] come back; host sums / B.
"""

import numpy as np

B = 32768
V = 256
N_CORES = 8
BL = B // N_CORES          # 4096 rows per core
P = 128                    # SBUF partitions
RPP = 8                    # rows per partition per DMA group
GROUP_ROWS = P * RPP       # 1024 rows per group (1 MB)
N_GROUPS = BL // GROUP_ROWS  # 4
NT = BL // P               # 32 selected values per partition
N_SQRT = 8
LOG_SCALE = float(1 << N_SQRT)  # 256
N_NEWTON = 2
PEN = 10.0                 # penalty scale sqrt: penalty = (PEN*d)^2

_CACHE = {}


def _build():
    from contextlib import ExitStack

    import concourse.bass as bass
    import concourse.tile as tile
    from concourse import bacc, mybir

    f32 = mybir.dt.float32
    bf16 = mybir.dt.bfloat16
    Alu = mybir.AluOpType
    Act = mybir.ActivationFunctionType

    nc = bacc.Bacc(
        "TRN2",
        target_bir_lowering=False,
        debug=False,
        enable_asserts=True,
        num_devices=N_CORES,
    )

    fs_d = nc.dram_tensor("fs", [BL, V], f32, kind="ExternalInput").ap()
    lab_d = nc.dram_tensor("labt", [P, NT], f32, kind="ExternalInput").ap()
    out_d = nc.dram_tensor("out", [P, 1], f32, kind="ExternalOutput").ap()

    with tile.TileContext(nc) as tc, ExitStack() as ctx:
        const_pool = ctx.enter_context(tc.tile_pool(name="const", bufs=1))
        fs_pool = ctx.enter_context(tc.tile_pool(name="fs", bufs=4))
        sq_pool = ctx.enter_context(tc.tile_pool(name="sq", bufs=8))
        prod_pool = ctx.enter_context(tc.tile_pool(name="prod", bufs=2))
        ep_pool = ctx.enter_context(tc.tile_pool(name="ep", bufs=2))

        iota = const_pool.tile([P, V], f32)
        nc.gpsimd.iota(
            iota[:],
            pattern=[[1, V]],
            base=0,
            channel_multiplier=0,
            allow_small_or_imprecise_dtypes=True,
        )

        labt = const_pool.tile([P, NT], f32)
        nc.sync.dma_start(out=labt[:], in_=lab_d[:])
        # bias for ACT Square: -PEN * lab per partition
        labp = const_pool.tile([P, NT], f32, tag="labp")
        nc.vector.tensor_scalar(labp[:], labt[:], -PEN, None, Alu.mult)

        sel = const_pool.tile([P, NT], f32, tag="sel")

        # fs rows g*1024 .. (g+1)*1024, partition p <- 8 consecutive rows
        fs_view = fs_d.rearrange("(g p j) v -> g p (j v)", g=N_GROUPS, p=P, j=RPP)

        for g in range(N_GROUPS):
            fst = fs_pool.tile([P, RPP * V], bf16)
            nc.gpsimd.dma_start(out=fst[:], in_=fs_view[g])
            prod = prod_pool.tile([P, RPP * V], bf16)
            for j in range(RPP):
                t = g * RPP + j
                # penalty = (PEN*iota - PEN*lab)^2 = PEN^2 * (v - lab)^2
                sq = sq_pool.tile([P, V], bf16)
                nc.scalar.activation(
                    sq[:],
                    iota[:],
                    Act.Square,
                    scale=PEN,
                    bias=labp[:, t : t + 1],
                )
                # t = fs - penalty  (== fs at v==lab, <= fs-88 elsewhere)
                nc.vector.tensor_sub(
                    prod[:, j * V : (j + 1) * V],
                    fst[:, j * V : (j + 1) * V],
                    sq[:],
                )
            # sel[:, g*8:(g+1)*8] = max over v of prod
            nc.vector.tensor_reduce(
                sel[:, g * RPP : (g + 1) * RPP],
                prod[:].rearrange("p (j v) -> p j v", j=RPP),
                axis=mybir.AxisListType.X,
                op=Alu.max,
            )

        # epilogue: y[p,t] = softplus(-sel[p,t]); out[p] = sum_t y[p,t]
        # u = exp(-sel); z = 1 + u
        u = ep_pool.tile([P, NT], f32, tag="u")
        nc.scalar.activation(u[:], sel[:], Act.Exp, scale=-1.0)
        z = ep_pool.tile([P, NT], f32, tag="z")
        nc.vector.tensor_scalar(z[:], u[:], 1.0, None, Alu.add)
        # initial guess: w = z^(1/256) via 8 sqrts; s = w-1;
        # y0 = 256 * s * (1 - s*(1/2 - s*(1/3 - s/4)))
        w = ep_pool.tile([P, NT], f32, tag="w")
        nc.scalar.activation(w[:], u[:], Act.Sqrt, bias=1.0)
        for _ in range(N_SQRT - 1):
            w2 = ep_pool.tile([P, NT], f32, tag="w")
            nc.scalar.activation(w2[:], w[:], Act.Sqrt)
            w = w2
        s = ep_pool.tile([P, NT], f32, tag="s")
        nc.vector.tensor_scalar(s[:], w[:], -1.0, None, Alu.add)
        c = ep_pool.tile([P, NT], f32, tag="c")
        nc.vector.tensor_scalar(c[:], s[:], -0.25, 1.0 / 3.0, Alu.mult, Alu.add)
        sc = ep_pool.tile([P, NT], f32, tag="sc")
        nc.vector.tensor_mul(sc[:], s[:], c[:])
        d = ep_pool.tile([P, NT], f32, tag="d")
        nc.vector.tensor_scalar(d[:], sc[:], -1.0, 0.5, Alu.mult, Alu.add)
        sd = ep_pool.tile([P, NT], f32, tag="sd")
        nc.vector.tensor_mul(sd[:], s[:], d[:])
        e = ep_pool.tile([P, NT], f32, tag="e")
        nc.vector.tensor_scalar(e[:], sd[:], -1.0, 1.0, Alu.mult, Alu.add)
        se = ep_pool.tile([P, NT], f32, tag="se")
        nc.vector.tensor_mul(se[:], s[:], e[:])
        y = ep_pool.tile([P, NT], f32, tag="y")
        nc.vector.tensor_scalar(y[:], se[:], LOG_SCALE, None, Alu.mult)
        # Newton: y' = y + z*exp(-y) - 1   (fixed point: exp(y)=z)
        for it in range(N_NEWTON):
            ey = ep_pool.tile([P, NT], f32, tag="ey")
            nc.scalar.activation(ey[:], y[:], Act.Exp, scale=-1.0)
            pz = ep_pool.tile([P, NT], f32, tag="pz")
            nc.vector.tensor_mul(pz[:], z[:], ey[:])
            q = ep_pool.tile([P, NT], f32, tag="q")
            nc.vector.tensor_add(q[:], y[:], pz[:])
            y2 = ep_pool.tile([P, NT], f32, tag="y")
            nc.vector.tensor_scalar(y2[:], q[:], -1.0, None, Alu.add)
            y = y2
        acc = ep_pool.tile([P, 1], f32, tag="acc")
        jep = ep_pool.tile([P, NT], f32, tag="jep")
        nc.scalar.activation(jep[:], y[:], Act.Copy, accum_out=acc[:])
        nc.sync.dma_start(out=out_d[:], in_=acc[:])

    nc.compile()
    return nc


def _get_nc():
    if "nc" not in _CACHE:
        _CACHE["nc"] = _build()
    return _CACHE["nc"]


def _shard_inputs(fs, labels):
    fs = np.ascontiguousarray(np.asarray(fs, dtype=np.float32))
    labels = np.asarray(labels)
    in_maps = []
    for c in range(N_CORES):
        fs_loc = fs[c * BL : (c + 1) * BL]
        lab_loc = labels[c * BL : (c + 1) * BL]
        # labt[p, g*RPP + j] = lab_loc[g*GROUP_ROWS + p*RPP + j]
        labt = (
            lab_loc.reshape(N_GROUPS, P, RPP)
            .transpose(1, 0, 2)
            .reshape(P, NT)
            .astype(np.float32)
        )
        in_maps.append({"fs": fs_loc, "labt": np.ascontiguousarray(labt)})
    return in_maps


def kernel(fs, labels, _trace=False, _trace_kwargs=None):
    from concourse.bass_utils import run_bass_kernel_spmd

    nc = _get_nc()
    in_maps = _shard_inputs(fs, labels)
    res = run_bass_kernel_spmd(
        nc,
        in_maps,
        core_ids=list(range(N_CORES)),
        trace=_trace,
        **(_trace_kwargs or {}),
    )
    total = np.float64(0.0)
    for c in range(N_CORES):
        total += res.results[c]["out"].astype(np.float64).sum()
    loss = total / np.float64(B)
    if _trace:
        return np.float64(loss), res
    return np.asarray(loss, dtype=np.float64)


# revision 7
# speedup vs baseline: 3.7747x; 1.0123x over previous
"""HEX loss kernel for Trainium2 (8 NeuronCores, batch-parallel, raw Bass).

Math: the chain junction-tree potential is rank-1 per clique and each
interior fs[v] is split fs[v]/2 over its two cliques, so the joint
distribution factorizes into independent Bernoullis with
P(y_v=1) = sigmoid(fs[b,v]); hence
    loss = mean_b softplus(-fs[b, labels[b]])
(verified to 1.4e-16 vs the f64 junction-tree reference).

Per core (4096 rows, pure data parallel): stream fs (4 MB) as 4x1MB
SWDGE cast-DMAs (f32->bf16). Exact gather via max-trick: ACT computes
penalty = Square(10*iota - 10*lab) per row-tile, DVE does one wide
[128,2048] subtract per group and a grouped reduce_max ->
sel = fs[b, lab]. softplus(-x) = ln(1+exp(-x)) is built from Exp only
(gen3 ACT tables have no Ln): exponent-bit log2 initial guess, then 2
Newton steps y += z*exp(-y) - 1. Host sums 8x128 partials / B.
"""

import numpy as np

B = 32768
V = 256
N_CORES = 8
BL = B // N_CORES
P = 128
RPP = 8
GROUP_ROWS = P * RPP       # 1024 rows, 1 MB f32
N_GROUPS = BL // GROUP_ROWS  # 4
NT = BL // P               # 32
N_SQRT = 8
LOG_SCALE = float(1 << N_SQRT)
N_NEWTON = 2
PEN = 10.0

_CACHE = {}


def _build():
    from contextlib import ExitStack

    import concourse.bass as bass
    import concourse.tile as tile  # noqa
    from concourse import bacc, mybir

    f32 = mybir.dt.float32
    bf16 = mybir.dt.bfloat16
    Alu = mybir.AluOpType
    Act = mybir.ActivationFunctionType

    nc = bacc.Bacc(
        "TRN2",
        target_bir_lowering=False,
        debug=False,
        enable_asserts=True,
        num_devices=N_CORES,
    )

    fs_d = nc.dram_tensor("fs", [BL, V], f32, kind="ExternalInput").ap()
    lab_d = nc.dram_tensor("labt", [P, NT], f32, kind="ExternalInput").ap()
    out_d = nc.dram_tensor("out", [P, 1], f32, kind="ExternalOutput").ap()

    fs_view = fs_d.rearrange("(g p j) v -> g p (j v)", g=N_GROUPS, p=P, j=RPP)

    with ExitStack() as ctx:
        # SBUF allocations
        iota = ctx.enter_context(nc.sbuf_tensor([P, V], f32))
        labt = ctx.enter_context(nc.sbuf_tensor([P, NT], f32))
        labp = ctx.enter_context(nc.sbuf_tensor([P, NT], f32))
        sel = ctx.enter_context(nc.sbuf_tensor([P, NT], f32))
        fs_t = [ctx.enter_context(nc.sbuf_tensor(f"fs_t{i}", [P, RPP * V], bf16)) for i in range(N_GROUPS)]
        sq_big = [ctx.enter_context(nc.sbuf_tensor(f"sq_big{i}", [P, RPP * V], bf16)) for i in range(2)]
        prod = [ctx.enter_context(nc.sbuf_tensor(f"prod{i}", [P, RPP * V], bf16)) for i in range(2)]
        # epilogue tiles
        u = ctx.enter_context(nc.sbuf_tensor([P, NT], f32))
        z = ctx.enter_context(nc.sbuf_tensor([P, NT], f32))
        w1 = ctx.enter_context(nc.sbuf_tensor([P, NT], f32))
        w2 = ctx.enter_context(nc.sbuf_tensor([P, NT], f32))
        sS = ctx.enter_context(nc.sbuf_tensor([P, NT], f32))
        tA = ctx.enter_context(nc.sbuf_tensor([P, NT], f32))
        tB = ctx.enter_context(nc.sbuf_tensor([P, NT], f32))
        yv = ctx.enter_context(nc.sbuf_tensor([P, NT], f32))
        ey = ctx.enter_context(nc.sbuf_tensor([P, NT], f32))
        jnk = ctx.enter_context(nc.sbuf_tensor([P, NT], f32))
        acc = ctx.enter_context(nc.sbuf_tensor([P, 1], f32))

        sem_iota = ctx.enter_context(nc.semaphore("s_iota"))
        sem_lab = ctx.enter_context(nc.semaphore("s_lab"))
        sem_labp = ctx.enter_context(nc.semaphore("s_labp"))
        sem_fs = [ctx.enter_context(nc.semaphore(f"s_fs{g}")) for g in range(N_GROUPS)]
        sem_sq = ctx.enter_context(nc.semaphore("s_sq"))
        sem_sub = ctx.enter_context(nc.semaphore("s_sub"))
        sem_red = ctx.enter_context(nc.semaphore("s_red"))
        sem_epa = ctx.enter_context(nc.semaphore("s_epa"))  # ACT -> DVE
        sem_epd = ctx.enter_context(nc.semaphore("s_epd"))  # DVE -> ACT
        sem_acc = ctx.enter_context(nc.semaphore("s_acc"))
        sem_out = ctx.enter_context(nc.semaphore("s_out"))

        blk = ctx.enter_context(nc.Block())

        @blk.gpsimd
        def _(g_eng):
            g_eng.iota(
                iota.ap(),
                pattern=[[1, V]],
                base=0,
                channel_multiplier=0,
                allow_small_or_imprecise_dtypes=True,
            ).then_inc(sem_iota, 1)
            for g in range(N_GROUPS):
                g_eng.dma_start(out=fs_t[g].ap(), in_=fs_view[g]).then_inc(
                    sem_fs[g], 16
                )

        @blk.sync
        def _(s_eng):
            s_eng.dma_start(out=labt.ap(), in_=lab_d).then_inc(sem_lab, 16)
            s_eng.wait_ge(sem_acc, 1)
            s_eng.dma_start(out=out_d, in_=acc.ap()).then_inc(sem_out, 16)
            s_eng.wait_ge(sem_out, 16)

        @blk.scalar
        def _(a_eng):
            a_eng.wait_ge(sem_iota, 1)
            a_eng.wait_ge(sem_labp, 1)
            for t in range(NT):
                g, j = t // RPP, t % RPP
                if j == 0 and g >= 2:
                    a_eng.wait_ge(sem_sub, g - 1)
                a_eng.activation(
                    sq_big[g % 2].ap()[:, j * V : (j + 1) * V],
                    iota.ap(),
                    Act.Square,
                    scale=PEN,
                    bias=labp.ap()[:, t : t + 1],
                ).then_inc(sem_sq, 1)
            # epilogue (ACT side)
            a_eng.wait_ge(sem_red, N_GROUPS)
            a_eng.activation(u.ap(), sel.ap(), Act.Exp, scale=-1.0).then_inc(
                sem_epa, 1
            )
            for i in range(N_NEWTON):
                a_eng.wait_ge(sem_epd, i + 1)
                a_eng.activation(ey.ap(), yv.ap(), Act.Exp, scale=-1.0).then_inc(
                    sem_epa, 1
                )

        @blk.vector
        def _(v_eng):
            v_eng.wait_ge(sem_lab, 16)
            v_eng.tensor_scalar(labp.ap(), labt.ap(), -PEN, None, Alu.mult).then_inc(
                sem_labp, 1
            )
            for g in range(N_GROUPS):
                v_eng.wait_ge(sem_fs[g], 16)
                v_eng.wait_ge(sem_sq, RPP * (g + 1))
                pr = prod[g % 2]
                v_eng.tensor_sub(
                    pr.ap(), fs_t[g].ap(), sq_big[g % 2].ap()
                ).then_inc(sem_sub, 1)
                v_eng.drain()
                v_eng.tensor_reduce(
                    sel.ap()[:, g * RPP : (g + 1) * RPP],
                    pr.ap().rearrange("p (j v) -> p j v", j=RPP),
                    axis=mybir.AxisListType.X,
                    op=Alu.max,
                ).then_inc(sem_red, 1)
            # epilogue (DVE side)
            v_eng.wait_ge(sem_epa, 1)
            v_eng.tensor_scalar(z.ap(), u.ap(), 1.0, None, Alu.add)
            v_eng.drain()
            # y0 = ln2 * (float(bitcast_i32(z)) / 2^23 - 127): log2 from the
            # exponent+mantissa bits, max abs err ~0.06 -- Newton polishes.
            v_eng.tensor_copy(tA.ap(), z.ap().bitcast(mybir.dt.int32))
            v_eng.drain()
            v_eng.tensor_scalar(
                yv.ap(), tA.ap(), 0.6931471805599453 / (1 << 23),
                -127.0 * 0.6931471805599453, Alu.mult, Alu.add,
            ).then_inc(sem_epd, 1)
            for i in range(N_NEWTON):
                v_eng.wait_ge(sem_epa, 2 + i)
                v_eng.tensor_mul(tB.ap(), z.ap(), ey.ap())
                v_eng.drain()
                v_eng.tensor_add(tA.ap(), yv.ap(), tB.ap())
                v_eng.drain()
                v_eng.tensor_scalar(yv.ap(), tA.ap(), -1.0, None, Alu.add).then_inc(
                    sem_epd, 1
                )
            v_eng.drain()
            v_eng.tensor_reduce(
                acc.ap(), yv.ap(), axis=mybir.AxisListType.X, op=Alu.add
            ).then_inc(sem_acc, 1)

    nc.compile()
    return nc


def _get_nc():
    if "nc" not in _CACHE:
        _CACHE["nc"] = _build()
    return _CACHE["nc"]


def _shard_inputs(fs, labels):
    fs = np.ascontiguousarray(np.asarray(fs, dtype=np.float32))
    labels = np.asarray(labels)
    in_maps = []
    for c in range(N_CORES):
        fs_loc = fs[c * BL : (c + 1) * BL]
        lab_loc = labels[c * BL : (c + 1) * BL]
        labt = (
            lab_loc.reshape(N_GROUPS, P, RPP)
            .transpose(1, 0, 2)
            .reshape(P, NT)
            .astype(np.float32)
        )
        in_maps.append({"fs": fs_loc, "labt": np.ascontiguousarray(labt)})
    return in_maps


def kernel(fs, labels, _trace=False, _trace_kwargs=None):
    from concourse.bass_utils import run_bass_kernel_spmd

    nc = _get_nc()
    in_maps = _shard_inputs(fs, labels)
    res = run_bass_kernel_spmd(
        nc,
        in_maps,
        core_ids=list(range(N_CORES)),
        trace=_trace,
        **(_trace_kwargs or {}),
    )
    total = np.float64(0.0)
    for c in range(N_CORES):
        total += res.results[c]["out"].astype(np.float64).sum()
    loss = total / np.float64(B)
    if _trace:
        return np.float64(loss), res
    return np.asarray(loss, dtype=np.float64)
